# revision 67
# baseline (speedup 1.0000x reference)
"""Policy-network kernel for Trainium2 (Bass/Tile), SPMD over 8 NeuronCores.

Strategy: data-parallel over batch B=128 -> 16 batches per core; all tables
and MLP weights replicated; no collectives. Heavy matmuls run in bf16.

Perf notes driving the structure:
- Every dma_start costs ~0.6us of serialized descriptor processing plus
  ~0.6us on the triggering engine's sequencer, and the DMA engines behave
  like one serial ~345GB/s resource, so all inputs are packed into a
  handful of wide bf16 blobs (f32 fields bit-packed as bf16 column pairs
  and bitcast on-chip) issued on one queue in consumption order.
- The per-(b,a) relation gather of the attention output is a one-hot matmul
  whose one-hot operand is precomputed on the host (scatter, cheap) and
  DMA'd, keeping the DVE free.
- h1 is computed as (W1b^T saq) @ alpha_g (associativity) which costs fewer
  PE cycles than W1b^T (saq @ alpha_g) and avoids transposing saq.
- The final logit dot fuses multiply+reduce into one DVE
  scalar_tensor_tensor (tensor_tensor_reduce fails at NEFF runtime).
- A burst of tiny warm-up matmuls keeps the PE clock ramped while the
  first input DMAs land; relu/copy work is split across Scalar and DVE so
  neither serializes the PE's h1->h2 chain.
"""

import numpy as np

# Problem dims (hardcoded per contract)
B, S, Dw, Dr, De, H, R, E, A = 128, 32, 300, 256, 256, 512, 512, 50000, 256
ACT = Dr + De          # 512
NCORES = 8
BL = B // NCORES       # 16 batches per core
BSL = BL * S           # 512 rows per core
NEG = -1e9

# ---- packed input blob column maps (bf16 columns; f32 fields use 2 cols) ----
# FRONT blob [128, FB]: copy1 = saq inputs, copy2 = attention + misc consts
WS_OFF = [k * 768 for k in range(3)]            # w_step k-tile, 256 cols each
XT_OFF = [k * 768 + 256 for k in range(3)]      # xT k-tile, 512 cols each
BSTEP_OFF = 2304                                # 2 tiles x 1 f32 -> 4 cols
F_CP1 = 2308
RELW_OFF = F_CP1                                # 2 tiles x 512
MASK_OFF = RELW_OFF + 1024                      # [1, 512] row 0
IDENT_OFF = MASK_OFF + 512                      # [128,128] f32 -> 256 cols
PAIR_OFF = IDENT_OFF + 256                      # [32, 32] f32 -> 64 cols
AMASK_OFF = PAIR_OFF + 64                       # [128, 32] f32 -> 64 cols
FB = AMASK_OFF + 64

# WB blob [128, WB]: copy1 = biasT inputs, copy2 = U/h2 weights
W1A_OFF = 0                                     # 4 x 512
PH_OFF = 2048                                   # 4 x 16
B1_OFF = PH_OFF + 64                            # 4 x 1 f32 -> 8 cols
W_CP1 = B1_OFF + 8
W1B_OFF = W_CP1                                 # 2 x 512
W2_OFF = W1B_OFF + 1024                         # 4 x 512
WB = W2_OFF + 2048

_CACHE = {}


def _build():
    import concourse.bass as bass
    import concourse.tile as tile
    from concourse import bacc, mybir

    f32 = mybir.dt.float32
    f32r = mybir.dt.float32r
    f8 = mybir.dt.float8e4
    bf16 = mybir.dt.bfloat16
    ts = bass.ts
    AF = mybir.ActivationFunctionType
    ALU = mybir.AluOpType

    nc = bacc.Bacc("TRN2", target_bir_lowering=False, debug=False)

    OHC = BL * 2 * A      # one-hot stored fp8: 2 bytes/bf16 col
    GC = 2 * BL * ACT
    cfg_in = nc.dram_tensor("cfg_in", [128, FB + WB], bf16, kind="ExternalInput").ap()
    bulk_in = nc.dram_tensor("bulk_in", [128, OHC + GC], bf16, kind="ExternalInput").ap()
    front_in = cfg_in[:, 0:FB]
    wb_in = cfg_in[:, FB:FB + WB]
    onehot_in = bulk_in[:, 0:OHC]
    g_all_in = bulk_in[:, OHC:OHC + GC]
    out_dram = nc.dram_tensor("out", [BL, A], f32, kind="ExternalOutput").ap()

    with tile.TileContext(nc) as tc:
        with (
            tc.tile_pool(name="const", bufs=1) as cpool,
            tc.tile_pool(name="work", bufs=2) as wpool,
            tc.tile_pool(name="perb", bufs=4) as bpool,
            tc.tile_pool(name="ps_big", bufs=2, space="PSUM") as ps_big,
            tc.tile_pool(name="ps_h2", bufs=2, space="PSUM") as ps_h2p,
            tc.tile_pool(name="ps_h1", bufs=2, space="PSUM") as ps_h1p,
            tc.tile_pool(name="ps_ag", bufs=2, space="PSUM") as ps_agp,
        ):
            # ---- inputs: 10 DMA copies total ----
            # synthesized constants first (DVE memsets: ~free, unblock act warm)
            ones1 = cpool.tile([1, 128], bf16, tag="ones1")
            nc.vector.memset(ones1[:], 1.0)
            ones_col = cpool.tile([128, 1], f32, tag="ones_col")
            nc.vector.memset(ones_col[:], 1.0)
            warm0 = cpool.tile([128, 1], f32, tag="warm0")
            nc.vector.memset(warm0[:], 0.0)
            front = cpool.tile([128, FB], bf16, tag="front")
            wb = cpool.tile([128, WB], bf16, tag="wb")
            g_all = cpool.tile([128, 2 * BL, ACT], bf16, tag="g_all")
            g_flat = g_all[:].rearrange("p j d -> p (j d)")
            oh = cpool.tile([128, BL, 2 * A], bf16, tag="oh")
            oh_flat = oh[:].rearrange("p b x -> p (b x)")
            # one queue, ordered by first consumption: the DMA engines are a
            # serial resource, so global order == arrival order matters most
            OCH = BL * 2 * A // 4
            CH4 = 2 * BL * ACT // 4
            F_CP2 = MASK_OFF + 512
            nc.sync.dma_start(front[:, 0:F_CP1], front_in[:, 0:F_CP1])
            nc.sync.dma_start(front[:, F_CP1:F_CP2], front_in[:, F_CP1:F_CP2])
            nc.sync.dma_start(front[:, F_CP2:FB], front_in[:, F_CP2:FB])
            nc.sync.dma_start(oh_flat[:, 0:OCH], onehot_in[:, 0:OCH])
            nc.sync.dma_start(wb[:, 0:W_CP1], wb_in[:, 0:W_CP1])
            nc.sync.dma_start(wb[:, W_CP1:W_CP1 + 1024], wb_in[:, W_CP1:W_CP1 + 1024])
            nc.sync.dma_start(oh_flat[:, OCH:2 * OCH], onehot_in[:, OCH:2 * OCH])
            nc.sync.dma_start(wb[:, W2_OFF:WB], wb_in[:, W2_OFF:WB])
            nc.sync.dma_start(g_flat[:, 0:CH4], g_all_in[:, 0:CH4])
            nc.sync.dma_start(g_flat[:, CH4:2 * CH4], g_all_in[:, CH4:2 * CH4])
            nc.sync.dma_start(oh_flat[:, 2 * OCH:3 * OCH], onehot_in[:, 2 * OCH:3 * OCH])
            nc.sync.dma_start(g_flat[:, 2 * CH4:3 * CH4], g_all_in[:, 2 * CH4:3 * CH4])
            nc.sync.dma_start(oh_flat[:, 3 * OCH:4 * OCH], onehot_in[:, 3 * OCH:4 * OCH])
            nc.sync.dma_start(g_flat[:, 3 * CH4:4 * CH4], g_all_in[:, 3 * CH4:4 * CH4])
            Elg = cpool.tile([128, 2 * BL], f32, tag="Elg")
            # warm the ACT function table immediately (set contains exp/tanh/relu)
            act_warm = cpool.tile([128, 1], f32, tag="act_warm")
            nc.scalar.activation(act_warm[:], warm0[:], AF.Exp)

            # views into the blobs
            w_step_sb = [front[:, WS_OFF[k]:WS_OFF[k] + 256] for k in range(3)]
            xT_sb = [front[:, XT_OFF[k]:XT_OFF[k] + 512] for k in range(3)]
            relwT_sb = [front[:, RELW_OFF + 512 * k:RELW_OFF + 512 * (k + 1)] for k in range(2)]
            b_step_sb = [front[:, BSTEP_OFF + 2 * k:BSTEP_OFF + 2 * k + 2].bitcast(f32) for k in range(2)]
            mask_sb = front[0:1, MASK_OFF:MASK_OFF + 512]
            ident_f = front[:, IDENT_OFF:IDENT_OFF + 256].bitcast(f32)
            pairmat_sb = front[0:32, PAIR_OFF:PAIR_OFF + 64].bitcast(f32)
            amask_p = front[:, AMASK_OFF:AMASK_OFF + 64].bitcast(f32)
            w1a_sb = [wb[:, W1A_OFF + 512 * k:W1A_OFF + 512 * (k + 1)] for k in range(4)]
            phT_sb = [wb[:, PH_OFF + 16 * k:PH_OFF + 16 * (k + 1)] for k in range(4)]
            b1_sb = [wb[:, B1_OFF + 2 * t:B1_OFF + 2 * t + 2].bitcast(f32) for t in range(4)]
            w1b_sb = [wb[:, W1B_OFF + 512 * k:W1B_OFF + 512 * (k + 1)] for k in range(2)]
            w2_sb = [wb[:, W2_OFF + 512 * k:W2_OFF + 512 * (k + 1)] for k in range(4)]

            # ---- PE warmup: keep the PE busy/ramped while input DMAs land ----
            warm_ps = ps_agp.tile([64, A], f32, tag="ag")
            for _ in range(48):
                nc.tensor.matmul(warm_ps[0:1, 0:64], ones1[:, 0:1], ones1[:, 0:64],
                                 start=True, stop=True)

            # ---- saqT = tanh(W_step.T @ xT + b_step)  [2][128, BSL] ----
            saqT_sb = []
            for t in range(2):
                ps = ps_big.tile([128, BSL], f32, tag="big")
                for k in range(3):
                    nc.tensor.matmul(ps[:], w_step_sb[k][:, ts(t, 128)], xT_sb[k],
                                     start=(k == 0), stop=(k == 2))
                sb = cpool.tile([128, BSL], bf16, tag=f"saqT{t}")
                nc.scalar.activation(sb[:], ps[:], AF.Tanh, bias=b_step_sb[t])
                saqT_sb.append(sb)

            # ---- scores + masked softmax per r-tile -> alpha [4][128, BL, S],
            #      with biasT matmuls interleaved to fill PE gaps ----
            biasT = cpool.tile([128, 4, BL], f32, tag="biasT")

            def emit_biasT(t):
                ps = ps_h1p.tile([128, A], f32, tag="h1")
                for k in range(4):
                    nc.tensor.matmul(ps[:, 0:BL], w1a_sb[k][:, ts(t, 128)], phT_sb[k],
                                     start=(k == 0), stop=(k == 3))
                nc.vector.tensor_scalar_add(biasT[:, t, :], ps[:, 0:BL], b1_sb[t])

            alpha_sb = []
            for rt in range(4):
                if rt % 2 == 0:
                    ps = ps_big.tile([128, BSL], f32, tag="big")
                else:
                    ps = ps_h2p.tile([128, BSL], f32, tag="h2")
                for k in range(2):
                    nc.tensor.matmul(ps[:], relwT_sb[k][:, ts(rt, 128)], saqT_sb[k][:],
                                     start=(k == 0), stop=False)
                nc.tensor.matmul(ps[:], ones1[:], mask_sb, start=False, stop=True)
                al32 = wpool.tile([128, BL, S], f32, tag="al32")
                nc.scalar.activation(al32[:].rearrange("p b s -> p (b s)"), ps[:], AF.Exp)
                sums = wpool.tile([128, BL], f32, tag="sums")
                nc.vector.tensor_reduce(sums[:], al32[:], axis=mybir.AxisListType.X,
                                        op=ALU.add)
                rec = wpool.tile([128, BL], f32, tag="rec")
                nc.vector.reciprocal(rec[:], sums[:])
                al = cpool.tile([128, BL, S], bf16, tag=f"alpha{rt}")
                nc.vector.tensor_mul(al[:], al32[:],
                                     rec[:].unsqueeze(2).to_broadcast((128, BL, S)))
                alpha_sb.append(al)
                if rt >= 1:
                    emit_biasT(rt - 1)
            emit_biasT(3)

            # ---- per-batch pipeline, stage-major over groups of 4 ----
            ag_sbs, U_sbs, h1Ts = {}, {}, {}
            for g in range(BL // 4):
                bs = range(4 * g, 4 * g + 4)
                # pair-batched gather + U: two batches share one [64, .] tile,
                # strips at partition offsets 0/32 (base must be 0/32/64)
                for p in range(2):
                    pb = 4 * g + 2 * p
                    ps_ag = ps_agp.tile([64, A], f32, tag="ag")
                    for i in range(2):
                        sl = slice(i * 32, i * 32 + 32)
                        for rt in range(4):
                            nc.tensor.matmul(ps_ag[sl, :], alpha_sb[rt][:, pb + i, :],
                                             oh[:, pb + i, rt * 128:(rt + 1) * 128].bitcast(f8),
                                             start=(rt == 0), stop=(rt == 3))
                    ag_sb = bpool.tile([64, A], bf16, tag="ag_sb", bufs=4)
                    nc.scalar.copy(ag_sb[:], ps_ag[:])
                    ag_sbs[pb] = ag_sb
                    # U[(i,s), m] = sum_d saq[d, (b,s)] W1b[d, m] for both batches
                    ps_U = ps_big.tile([128, BSL], f32, tag="big")
                    for t in range(2):
                        nc.tensor.matmul(ps_U[0:64, :], saqT_sb[t][:, pb * S:(pb + 2) * S],
                                         w1b_sb[t], start=(t == 0), stop=(t == 1))
                    U_sb = bpool.tile([64, ACT], bf16, tag="U_sb", bufs=4)
                    nc.scalar.copy(U_sb[:], ps_U[0:64, :])
                    U_sbs[pb] = U_sb
                for b in bs:
                    # h1[m, a] = relu(sum_s U[s, m] ag[s, a] + biasT[m, b])
                    sl = slice((b % 2) * 32, (b % 2) * 32 + 32)
                    h1T = bpool.tile([128, 4, A], bf16, tag="h1T", bufs=8)
                    for t in range(4):
                        ps_h1 = ps_h1p.tile([128, A], f32, tag="h1")
                        nc.tensor.matmul(ps_h1[:], U_sbs[b - b % 2][sl, ts(t, 128)],
                                         ag_sbs[b - b % 2][sl, :],
                                         start=True, stop=True)
                        if t < 1 or g >= BL // 4 - 2:
                            nc.scalar.activation(h1T[:, t, :], ps_h1[:], AF.Relu,
                                                 bias=biasT[:, t, b:b + 1])
                        else:
                            nc.vector.tensor_scalar(h1T[:, t, :], ps_h1[:],
                                                    biasT[:, t, b:b + 1], 0.0,
                                                    op0=ALU.add, op1=ALU.max)
                    h1Ts[b] = h1T
                for b in bs:
                    for c in range(2):
                        ps_h2 = ps_h2p.tile([128, ACT], f32, tag="h2")
                        for k in range(4):
                            nc.tensor.matmul(ps_h2[:], h1Ts[b][:, k, ts(c, 128)], w2_sb[k],
                                             start=(k == 0), stop=(k == 3))
                        # logits[j] = sum_act h2 * g (fused mul+reduce on DVE);
                        # the action mask is added before the final exp
                        j = b * 2 + c
                        trash = bpool.tile([128, ACT], bf16, tag="trash", bufs=2)
                        nc.vector.scalar_tensor_tensor(
                            trash[:], ps_h2[:], 1.0, g_all[:, j, :],
                            op0=ALU.mult, op1=ALU.mult,
                            accum_out=Elg[:, j:j + 1])
            # ---- final softmax fully on-chip, partition layout ----
            Elgm = wpool.tile([128, 2 * BL], f32, tag="Elgm")
            nc.gpsimd.tensor_add(Elgm[:], Elg[:], amask_p)
            Eexp = wpool.tile([128, 2 * BL], f32, tag="Eexp")
            nc.scalar.activation(Eexp[:], Elgm[:], AF.Exp)
            ps_ET = ps_h1p.tile([128, A], f32, tag="h1")
            nc.tensor.transpose(ps_ET[0:32, 0:128], Eexp[:], ident_f)
            ps_s = ps_agp.tile([64, A], f32, tag="ag")
            nc.tensor.matmul(ps_s[0:32, 0:1], Eexp[:], ones_col[:], start=True, stop=True)
            s_sb = wpool.tile([32, 1], f32, tag="s_sb")
            nc.vector.tensor_copy(s_sb[:], ps_s[0:32, 0:1])
            ps_s2 = ps_agp.tile([64, A], f32, tag="ag")
            nc.tensor.matmul(ps_s2[0:32, 0:1], pairmat_sb, s_sb[:], start=True, stop=True)
            rec_sb = wpool.tile([32, 1], f32, tag="rec_sb")
            nc.vector.reciprocal(rec_sb[:], ps_s2[0:32, 0:1])
            osb = wpool.tile([32, 128], f32, tag="osb")
            nc.vector.tensor_scalar_mul(osb[:], ps_ET[0:32, 0:128], rec_sb[:])
            nc.sync.dma_start(out_dram[:].rearrange("b (c p) -> (b c) p", c=2), osb[:])
    nc.compile()
    return nc


def _host_prep(inputs):
    """Build the 8 per-core input maps from full inputs."""
    import ml_dtypes
    BF = ml_dtypes.bfloat16
    F8 = ml_dtypes.float8_e4m3

    x = np.asarray(inputs["transformer_output"], np.float32)
    qmask = np.asarray(inputs["question_mask"])
    W_step = np.ascontiguousarray(np.asarray(inputs["W_step"], np.float32))
    b_step = np.asarray(inputs["b_step"], np.float32).reshape(Dr, 1)
    w_att = np.asarray(inputs["w_att"], np.float32)
    rel_emb = np.ascontiguousarray(np.asarray(inputs["rel_emb"], np.float32))
    ent_emb = np.ascontiguousarray(np.asarray(inputs["ent_emb"], np.float32))
    ph = np.asarray(inputs["path_hidden"], np.float32)
    W1 = np.asarray(inputs["W1"], np.float32)
    b1 = np.asarray(inputs["b1"], np.float32).reshape(ACT, 1)
    W2 = np.ascontiguousarray(np.asarray(inputs["W2"], np.float32))
    b2 = np.asarray(inputs["b2"], np.float32).reshape(1, ACT)
    r_space = np.asarray(inputs["r_space"], np.int32)
    e_space = np.asarray(inputs["e_space"], np.int32)
    action_mask = np.asarray(inputs["action_mask"], np.float32)

    relwT = np.ascontiguousarray((rel_emb * w_att[None, :]).T)   # [Dr, R]
    w1a = np.ascontiguousarray(W1[:H])
    w1b = np.ascontiguousarray(W1[H:])
    rel_bf = np.ascontiguousarray(rel_emb.astype(BF))
    ent_bf = np.ascontiguousarray(ent_emb.astype(BF))
    c_rel = rel_emb @ b2[0, :Dr]
    c_ent = ent_emb @ b2[0, Dr:]
    jj = np.arange(2 * BL)
    pairmat_np = (jj[:, None] // 2 == jj[None, :] // 2).astype(np.float32)
    ident_np = np.eye(128, dtype=np.float32)

    def put_bf(dst, c0, arr):
        a = np.asarray(arr, np.float32).astype(BF).view(np.uint16)
        dst[:a.shape[0], c0:c0 + a.shape[1]] = a

    def put_f32(dst, c0, arr):
        a = np.ascontiguousarray(np.asarray(arr, np.float32))
        u = a.view(np.uint16)  # [rows, 2*cols]
        dst[:u.shape[0], c0:c0 + u.shape[1]] = u

    # WB blob is batch-independent: build once
    wb_u = np.zeros((128, WB), np.uint16)
    for k in range(4):
        put_bf(wb_u, W1A_OFF + 512 * k, w1a[128 * k:128 * (k + 1)])
        put_f32(wb_u, B1_OFF + 2 * k, b1[128 * k:128 * (k + 1)])
        put_bf(wb_u, W2_OFF + 512 * k, W2[128 * k:128 * (k + 1)])
    for k in range(2):
        put_bf(wb_u, W1B_OFF + 512 * k, w1b[128 * k:128 * (k + 1)])
    wb_base = wb_u

    bb = np.arange(BL)[:, None]
    aa = np.arange(A)[None, :]
    in_maps = []
    for i in range(NCORES):
        b0, b1_ = i * BL, (i + 1) * BL
        xs = x[b0:b1_].reshape(BSL, Dw)
        xT_np = xs.T                                      # [Dw, BSL]
        phT_np = np.ascontiguousarray(ph[b0:b1_].T)       # [H, BL]
        wb_u = wb_base.copy()
        for k in range(4):
            put_bf(wb_u, PH_OFF + 16 * k, phT_np[128 * k:128 * (k + 1)])
        mrow = np.where(qmask[b0:b1_].reshape(1, BSL), np.float32(NEG), np.float32(0.0))
        amask_add = np.where(action_mask[b0:b1_] > 0, np.float32(0.0), np.float32(NEG))
        amask_add = amask_add + c_rel[r_space[b0:b1_]] + c_ent[e_space[b0:b1_]]
        amask_p = np.ascontiguousarray(
            amask_add.reshape(BL, 2, 128).transpose(2, 0, 1).reshape(128, 2 * BL))

        front_u = np.zeros((128, FB), np.uint16)
        for k in range(3):
            put_bf(front_u, WS_OFF[k], W_step[128 * k:128 * (k + 1)])
            put_bf(front_u, XT_OFF[k], xT_np[128 * k:128 * (k + 1)])
        for k in range(2):
            put_bf(front_u, RELW_OFF + 512 * k, relwT[128 * k:128 * (k + 1)])
            put_f32(front_u, BSTEP_OFF + 2 * k, b_step[128 * k:128 * (k + 1)])
        put_bf(front_u, MASK_OFF, mrow)
        put_f32(front_u, IDENT_OFF, ident_np)
        put_f32(front_u, PAIR_OFF, pairmat_np)
        put_f32(front_u, AMASK_OFF, amask_p)

        grel = rel_bf[r_space[b0:b1_]]
        gent = ent_bf[e_space[b0:b1_]]
        g_np = np.empty((128, 2 * BL, ACT), BF)
        g_np[:, :, :Dr] = grel.reshape(BL, 2, 128, Dr).transpose(2, 0, 1, 3).reshape(128, 2 * BL, Dr)
        g_np[:, :, Dr:] = gent.reshape(BL, 2, 128, De).transpose(2, 0, 1, 3).reshape(128, 2 * BL, De)
        g_np = np.ascontiguousarray(g_np.reshape(128, 2 * BL * ACT))
        # one-hot gather matrix (fp8): oh[p, b, rt, a] = (r_space[b,a] == rt*128+p)
        r_c = r_space[b0:b1_]
        oh_np = np.zeros((128, BL, 4, A), F8)
        oh_np[(r_c % 128), bb, (r_c // 128), aa] = np.float32(1.0)
        oh_np = np.ascontiguousarray(oh_np.reshape(128, BL * 4 * A)).view(np.uint16).view(BF)
        cfg_u = np.concatenate([front_u, wb_u], axis=1)
        bulk_np = np.concatenate([oh_np, g_np], axis=1)
        in_maps.append({
            "cfg_in": cfg_u.view(BF),
            "bulk_in": bulk_np,
            # debug-only copies for test harnesses (ignored by the runner)
            "_r_space": r_c,
            "_xT": xT_np,
            "_w_step": W_step,
            "_b_step": b_step,
            "_relwT": relwT,
            "_mask_row": mrow,
            "_w1a": w1a,
            "_phT": phT_np,
            "_w1b": w1b,
            "_b1": b1,
            "_w2": W2,
            "_amask": amask_p,
        })
    return in_maps


def kernel(**inputs):
    from concourse.bass_utils import run_bass_kernel_spmd

    if "nc" not in _CACHE:
        _CACHE["nc"] = _build()
    nc = _CACHE["nc"]
    in_maps = _host_prep(inputs)
    res = run_bass_kernel_spmd(nc, in_maps, list(range(NCORES)))
    return np.concatenate([res.results[i]["out"] for i in range(NCORES)], axis=0)
